# revision 3
# baseline (speedup 1.0000x reference)
"""Per-pixel adaptive 5x5 conv (KPN apply) on 8 Trainium2 NeuronCores.

out[b,c,h,w] = sum_{i,j} core[b,0,i*5+j,c,h,w] * frames[b,0,c,h+i-2,w+j-2]
(zero-padded borders), output [4,3,512,512] f32.

Sharding: pure data parallel, core k -> (b = k//2, H-half = k%2).

The metric is dominated by per-run host<->device transfer of the 315MB
`core` tensor, so inputs are compressed on the host before shipping:
  - core   -> int8, symmetric quantization with scale s = min(amax, 4*std)/127
             (the scale is folded into the frames, so the device kernel is
             just sum(q_t * f'_t));
  - frames -> bf16 (pre-scaled by s, zero-padded with halo rows host-side);
  - out    -> bf16 on device, upcast to f32 host-side.
Measured end-to-end rel err of this scheme vs the f32 reference: ~9.7e-3.

Device kernel (raw bass; this walrus build only allows one semaphore wait
per compute/DMA instruction so Tile auto-sync is unavailable): per
128-row block, one DMA brings the 25 tap planes [128, 25*512] int8
(contiguous 12.8KB rows), one DMA brings a 5-row overlapping window of
the padded bf16 frame [128, 5, 516]; DVE does ONE mixed-dtype
(int8 x bf16 -> f32) multiply over a 4D access pattern covering all 25
taps, then 6 contiguous f32 tree-adds and a f32->bf16 copy of the result
row. Double-buffered loads/stores on the SP HWDGE FIFO as in the
baseline; DVE instructions on the same engine execute in order, so the
intra-block chain needs no semaphores.
"""

import ml_dtypes
import numpy as np

import concourse.bass as bass
import concourse.mybir as mybir
from concourse.ap import AP
from concourse.bass_utils import run_bass_kernel_spmd

B, N, C, H, W = 4, 1, 3, 512, 512
K = 5
PAD = K // 2
NCORES = 8
HH = H // (NCORES // B)   # 256 rows per core
P = 128
NBLK_TOT = C * (HH // P)  # 6 blocks of 128 rows per core
WPAD = W + 2 * PAD        # 516
HHP = HH + 2 * PAD        # 260
TW = K * K * W            # 12800 elements per row: all 25 taps

BF16 = ml_dtypes.bfloat16

_CACHE = {}


def _build():
    nc = bass.Bass()
    f32 = mybir.dt.float32
    bf16 = mybir.dt.bfloat16
    i8 = mybir.dt.int8

    fr = nc.declare_dram_parameter("fr", [C, HHP, WPAD], bf16, isOutput=False)
    co = nc.declare_dram_parameter("co", [C, HH, TW], i8, isOutput=False)
    out = nc.declare_dram_parameter("out", [C, HH, W], bf16, isOutput=True)

    def co_view(n):
        c, blk = n // (HH // P), n % (HH // P)
        return co[c, blk * P:blk * P + P, :]

    def fr_win(n):
        c, blk = n // (HH // P), n % (HH // P)
        fb = fr[c, blk * P:blk * P + P, :]
        return AP(fb.tensor, fb.offset, [(WPAD, P), (WPAD, K), (1, WPAD)])

    def out_view(n):
        c, blk = n // (HH // P), n % (HH // P)
        return out[c, blk * P:blk * P + P, :]

    with (
        nc.sbuf_tensor("ct0", [P, K, K, W], i8) as ct0,
        nc.sbuf_tensor("ct1", [P, K, K, W], i8) as ct1,
        nc.sbuf_tensor("ft0", [P, K, WPAD], bf16) as ft0,
        nc.sbuf_tensor("ft1", [P, K, WPAD], bf16) as ft1,
        nc.sbuf_tensor("prod", [P, TW], f32) as prod,
        nc.sbuf_tensor("ob0", [P, W], bf16) as ob0,
        nc.sbuf_tensor("ob1", [P, W], bf16) as ob1,
        nc.semaphore("dsem") as dsem,   # load completions (+16 per DMA)
        nc.semaphore("osem") as osem,   # store completions (+16 per DMA)
        nc.semaphore("vsem") as vsem,   # DVE per-block completion (+1)
        nc.Block() as block,
    ):
        cts, fts, obs = [ct0, ct1], [ft0, ft1], [ob0, ob1]

        @block.sync
        def _(sync: bass.BassEngine):
            for n in range(NBLK_TOT):
                if n >= 2:
                    # DVE done with block n-2 => its ct/ft buffers reusable,
                    # and ob[n-2] ready to store.
                    sync.wait_ge(vsem, n - 1)
                    sync.dma_start(
                        out=out_view(n - 2), in_=obs[n % 2][:]
                    ).then_inc(osem, 16)
                sync.dma_start(out=cts[n % 2][:], in_=co_view(n)).then_inc(dsem, 16)
                sync.dma_start(out=fts[n % 2][:], in_=fr_win(n)).then_inc(dsem, 16)
            sync.wait_ge(vsem, NBLK_TOT - 1)
            sync.dma_start(
                out=out_view(NBLK_TOT - 2), in_=obs[NBLK_TOT % 2][:]
            ).then_inc(osem, 16)
            sync.wait_ge(vsem, NBLK_TOT)
            sync.dma_start(
                out=out_view(NBLK_TOT - 1), in_=obs[(NBLK_TOT + 1) % 2][:]
            ).then_inc(osem, 16)
            sync.wait_ge(osem, 16 * NBLK_TOT)

        @block.vector
        def _(vector: bass.BassEngine):
            p0 = prod[:]
            pap = AP(p0.tensor, p0.offset,
                     [tuple(p0.ap[0]), (K * W, K), (W, K), (1, W)])
            for n in range(NBLK_TOT):
                ct, ft, ob = cts[n % 2], fts[n % 2], obs[n % 2]
                vector.wait_ge(dsem, 32 * (n + 1))
                if n >= 2:
                    # store of block n-2 (same ob buffer) must be done
                    vector.wait_ge(osem, 16 * (n - 1))
                f0 = ft[:, 0, 0:W]
                fap = AP(f0.tensor, f0.offset,
                         [tuple(f0.ap[0]), (WPAD, K), (1, K), (1, W)])
                # prod[p, i*5*W + j*W + w] = ct[p,i,j,w] * fr[p+i, j+w]
                vector.tensor_tensor(out=pap, in0=ct[:], in1=fap,
                                     op=mybir.AluOpType.mult)
                # tree-reduce the 25 tap planes (f32, contiguous slices)
                vector.tensor_add(out=prod[:, 0:12 * W],
                                  in0=prod[:, 0:12 * W],
                                  in1=prod[:, 12 * W:24 * W])
                vector.tensor_add(out=prod[:, 0:6 * W],
                                  in0=prod[:, 0:6 * W],
                                  in1=prod[:, 6 * W:12 * W])
                vector.tensor_add(out=prod[:, 0:3 * W],
                                  in0=prod[:, 0:3 * W],
                                  in1=prod[:, 3 * W:6 * W])
                vector.tensor_add(out=prod[:, 0:W],
                                  in0=prod[:, 0:W], in1=prod[:, W:2 * W])
                vector.tensor_add(out=prod[:, 0:W],
                                  in0=prod[:, 0:W], in1=prod[:, 2 * W:3 * W])
                vector.tensor_add(out=prod[:, 0:W],
                                  in0=prod[:, 0:W], in1=prod[:, 24 * W:25 * W])
                vector.tensor_copy(out=ob[:], in_=prod[:, 0:W]).then_inc(vsem, 1)
    return nc


def get_nc():
    if "nc" not in _CACHE:
        _CACHE["nc"] = _build()
    return _CACHE["nc"]


def shard_inputs(frames, core):
    frames = np.asarray(frames, dtype=np.float32)
    core = np.asarray(core, dtype=np.float32)
    # sampled std is plenty for picking the clip threshold (~3M samples)
    sd = float(core.ravel()[::101].std())
    amax = float(max(core.max(), -core.min()))
    s = min(amax, 4.0 * sd) / 127.0 if amax > 0 else 1.0
    inv = np.float32(1.0 / s)
    frs = (frames * np.float32(s)).astype(BF16)  # scale folded into frames
    in_maps = []
    for k in range(NCORES):
        b, half = k // 2, k % 2
        h0 = half * HH
        frp = np.zeros((C, HHP, WPAD), BF16)
        lo, hi = h0 - PAD, h0 + HH + PAD
        clo, chi = max(lo, 0), min(hi, H)
        frp[:, clo - lo:clo - lo + chi - clo, PAD:PAD + W] = frs[b, 0, :, clo:chi, :]
        # [25, C, HH, W] -> [C, HH, 25, W], quantize to int8
        sl = core[b, 0, :, :, h0:h0 + HH, :].transpose(1, 2, 0, 3)
        qt = sl * inv
        np.rint(qt, out=qt)
        np.clip(qt, -127, 127, out=qt)
        coq = np.ascontiguousarray(qt.astype(np.int8)).reshape(C, HH, TW)
        in_maps.append({"fr": frp, "co": coq})
    return in_maps


def run(in_maps, **kwargs):
    return run_bass_kernel_spmd(get_nc(), in_maps, list(range(NCORES)), **kwargs)


def kernel(frames, core):
    in_maps = shard_inputs(frames, core)
    res = run(in_maps).results
    outp = np.empty((B, C, H, W), np.float32)
    for k in range(NCORES):
        b, half = k // 2, k % 2
        outp[b, :, half * HH:(half + 1) * HH, :] = res[k]["out"]
    return outp


# revision 7
# speedup vs baseline: 1.0016x; 1.0016x over previous
"""Per-pixel adaptive 5x5 conv (KPN apply) on 8 Trainium2 NeuronCores.

out[b,c,h,w] = sum_{i,j} core[b,0,i*5+j,c,h,w] * frames[b,0,c,h+i-2,w+j-2]
(zero-padded borders), output [4,3,512,512] f32.

Sharding: pure data parallel, core k -> (b = k//2, H-half = k%2).

The metric is dominated by per-run host<->device transfer of the 315MB
`core` tensor, so inputs are compressed on the host before shipping:
  - core   -> int8, symmetric quantization with scale s = min(amax, 4*std)/127
             (the scale is folded into the frames, so the device kernel is
             just sum(q_t * f'_t));
  - frames -> bf16 (pre-scaled by s, zero-padded with halo rows host-side);
  - out    -> bf16 on device, upcast to f32 host-side.
Measured end-to-end rel err of this scheme vs the f32 reference: ~9.7e-3.

Device kernel (raw bass; this walrus build only allows one semaphore wait
per compute/DMA instruction so Tile auto-sync is unavailable): per
128-row block, one DMA brings the 25 tap planes [128, 25*512] int8
(contiguous 12.8KB rows), one DMA brings a 5-row overlapping window of
the padded bf16 frame [128, 5, 516]. The Act engine upcasts the int8
taps to bf16 (in parallel with DVE work on the previous block), then the
DVE runs entirely in bf16 2x mode: ONE multiply over a 4D access pattern
covering all 25 taps, 6 contiguous tree-adds, and a copy of the result
row. Double-buffered loads/stores on the SP HWDGE FIFO; instructions on
the same engine execute in order, so intra-block chains need no
semaphores. Modeled exec ~115us/core; the run is dominated by shipping
the 91MB of compressed inputs to the 8 cores.
"""

import ml_dtypes
import numpy as np

import concourse.bass as bass
import concourse.mybir as mybir
from concourse.ap import AP
from concourse.bass_utils import run_bass_kernel_spmd

B, N, C, H, W = 4, 1, 3, 512, 512
K = 5
PAD = K // 2
NCORES = 8
HH = H // (NCORES // B)   # 256 rows per core
P = 128
NBLK_TOT = C * (HH // P)  # 6 blocks of 128 rows per core
WPAD = W + 2 * PAD        # 516
HHP = HH + 2 * PAD        # 260
TW = K * K * W            # 12800 elements per row: all 25 taps

BF16 = ml_dtypes.bfloat16

_CACHE = {}


def _build():
    nc = bass.Bass()
    bf16 = mybir.dt.bfloat16
    i8 = mybir.dt.int8

    fr = nc.declare_dram_parameter("fr", [C, HHP, WPAD], bf16, isOutput=False)
    co = nc.declare_dram_parameter("co", [C, HH, TW], i8, isOutput=False)
    out = nc.declare_dram_parameter("out", [C, HH, W], bf16, isOutput=True)

    def co_view(n):
        c, blk = n // (HH // P), n % (HH // P)
        return co[c, blk * P:blk * P + P, :]

    def fr_win(n):
        c, blk = n // (HH // P), n % (HH // P)
        fb = fr[c, blk * P:blk * P + P, :]
        return AP(fb.tensor, fb.offset, [(WPAD, P), (WPAD, K), (1, WPAD)])

    def out_view(n):
        c, blk = n // (HH // P), n % (HH // P)
        return out[c, blk * P:blk * P + P, :]

    with (
        nc.sbuf_tensor("ct0", [P, K, K, W], i8) as ct0,
        nc.sbuf_tensor("ct1", [P, K, K, W], i8) as ct1,
        nc.sbuf_tensor("cb0", [P, K, K, W], bf16) as cb0,
        nc.sbuf_tensor("cb1", [P, K, K, W], bf16) as cb1,
        nc.sbuf_tensor("ft0", [P, K, WPAD], bf16) as ft0,
        nc.sbuf_tensor("ft1", [P, K, WPAD], bf16) as ft1,
        nc.sbuf_tensor("prod", [P, TW], bf16) as prod,
        nc.sbuf_tensor("ob0", [P, W], bf16) as ob0,
        nc.sbuf_tensor("ob1", [P, W], bf16) as ob1,
        nc.semaphore("dsem") as dsem,   # load completions (+16 per DMA)
        nc.semaphore("osem") as osem,   # store completions (+16 per DMA)
        nc.semaphore("asem") as asem,   # Act cast per-block completion (+1)
        nc.semaphore("vsem") as vsem,   # DVE per-block completion (+1)
        nc.Block() as block,
    ):
        cts, cbs, fts, obs = [ct0, ct1], [cb0, cb1], [ft0, ft1], [ob0, ob1]

        @block.sync
        def _(sync: bass.BassEngine):
            for n in range(NBLK_TOT):
                if n >= 2:
                    # DVE done with block n-2 => its ct/ft buffers reusable,
                    # and ob[n-2] ready to store.
                    sync.wait_ge(vsem, n - 1)
                    sync.dma_start(
                        out=out_view(n - 2), in_=obs[n % 2][:]
                    ).then_inc(osem, 16)
                sync.dma_start(out=cts[n % 2][:], in_=co_view(n)).then_inc(dsem, 16)
                sync.dma_start(out=fts[n % 2][:], in_=fr_win(n)).then_inc(dsem, 16)
            sync.wait_ge(vsem, NBLK_TOT - 1)
            sync.dma_start(
                out=out_view(NBLK_TOT - 2), in_=obs[NBLK_TOT % 2][:]
            ).then_inc(osem, 16)
            sync.wait_ge(vsem, NBLK_TOT)
            sync.dma_start(
                out=out_view(NBLK_TOT - 1), in_=obs[(NBLK_TOT + 1) % 2][:]
            ).then_inc(osem, 16)
            sync.wait_ge(osem, 16 * NBLK_TOT)

        @block.scalar
        def _(scalar: bass.BassEngine):
            for n in range(NBLK_TOT):
                scalar.wait_ge(dsem, 32 * n + 16)   # ct(n) loaded
                if n >= 2:
                    scalar.wait_ge(vsem, n - 1)     # mul(n-2) freed cb[n%2]
                scalar.copy(out=cbs[n % 2][:], in_=cts[n % 2][:]).then_inc(asem, 1)

        @block.vector
        def _(vector: bass.BassEngine):
            p0 = prod[:]
            pap = AP(p0.tensor, p0.offset,
                     [tuple(p0.ap[0]), (K * W, K), (W, K), (1, W)])
            for n in range(NBLK_TOT):
                cb, ft, ob = cbs[n % 2], fts[n % 2], obs[n % 2]
                vector.wait_ge(dsem, 32 * (n + 1))  # ft(n) loaded
                vector.wait_ge(asem, n + 1)         # cast(n) done
                if n >= 2:
                    # store of block n-2 (same ob buffer) must be done
                    vector.wait_ge(osem, 16 * (n - 1))
                f0 = ft[:, 0, 0:W]
                fap = AP(f0.tensor, f0.offset,
                         [tuple(f0.ap[0]), (WPAD, K), (1, K), (1, W)])
                # prod[p, i*5*W + j*W + w] = cb[p,i,j,w] * fr[p+i, j+w]
                vector.tensor_tensor(out=pap, in0=cb[:], in1=fap,
                                     op=mybir.AluOpType.mult)
                # tree-reduce the 25 tap planes (f32, contiguous slices)
                vector.tensor_add(out=prod[:, 0:12 * W],
                                  in0=prod[:, 0:12 * W],
                                  in1=prod[:, 12 * W:24 * W])
                vector.tensor_add(out=prod[:, 0:6 * W],
                                  in0=prod[:, 0:6 * W],
                                  in1=prod[:, 6 * W:12 * W])
                vector.tensor_add(out=prod[:, 0:3 * W],
                                  in0=prod[:, 0:3 * W],
                                  in1=prod[:, 3 * W:6 * W])
                vector.tensor_add(out=prod[:, 0:W],
                                  in0=prod[:, 0:W], in1=prod[:, W:2 * W])
                vector.tensor_add(out=prod[:, 0:W],
                                  in0=prod[:, 0:W], in1=prod[:, 2 * W:3 * W])
                vector.tensor_add(out=prod[:, 0:W],
                                  in0=prod[:, 0:W], in1=prod[:, 24 * W:25 * W])
                vector.tensor_copy(out=ob[:], in_=prod[:, 0:W]).then_inc(vsem, 1)
    return nc


def get_nc():
    if "nc" not in _CACHE:
        _CACHE["nc"] = _build()
    return _CACHE["nc"]


def shard_inputs(frames, core):
    frames = np.asarray(frames, dtype=np.float32)
    core = np.asarray(core, dtype=np.float32)
    # sampled std is plenty for picking the clip threshold (~3M samples)
    sd = float(core.ravel()[::101].std())
    amax = float(max(core.max(), -core.min()))
    s = min(amax, 4.0 * sd) / 127.0 if amax > 0 else 1.0
    inv = np.float32(1.0 / s)
    frs = (frames * np.float32(s)).astype(BF16)  # scale folded into frames
    in_maps = []
    for k in range(NCORES):
        b, half = k // 2, k % 2
        h0 = half * HH
        frp = np.zeros((C, HHP, WPAD), BF16)
        lo, hi = h0 - PAD, h0 + HH + PAD
        clo, chi = max(lo, 0), min(hi, H)
        frp[:, clo - lo:clo - lo + chi - clo, PAD:PAD + W] = frs[b, 0, :, clo:chi, :]
        # [25, C, HH, W] -> [C, HH, 25, W], quantize to int8
        sl = core[b, 0, :, :, h0:h0 + HH, :].transpose(1, 2, 0, 3)
        qt = sl * inv
        np.rint(qt, out=qt)
        np.clip(qt, -127, 127, out=qt)
        coq = np.ascontiguousarray(qt.astype(np.int8)).reshape(C, HH, TW)
        in_maps.append({"fr": frp, "co": coq})
    return in_maps


def run(in_maps, **kwargs):
    return run_bass_kernel_spmd(get_nc(), in_maps, list(range(NCORES)), **kwargs)


def kernel(frames, core):
    in_maps = shard_inputs(frames, core)
    res = run(in_maps).results
    outp = np.empty((B, C, H, W), np.float32)
    for k in range(NCORES):
        b, half = k // 2, k % 2
        outp[b, :, half * HH:(half + 1) * HH, :] = res[k]["out"]
    return outp
